# revision 17
# baseline (speedup 1.0000x reference)
"""DualGNN (2x [GCN->BN->ReLU]x2 -> mean-pool -> MLP head) on 8 trn2 NeuronCores.

Strategy
-------
Graphs are data-parallel: core k owns graphs [k*Gpc, (k+1)*Gpc) (batch is
sorted -> contiguous node ranges).  Edges are uniformly random over all nodes,
so each layer gathers rows of a replicated node-feature table.

Linear layers commute with aggregation, so W is applied per *node* before
gathering: the gather tables hold (dinv * h) @ W rows (bf16, padded to
256B rows for dma_gather).  Per core, per layer:
  - table section build: per-128-slot tile: scale by dinv, PE-transpose,
    matmul W, cast bf16, write to this core's part; AllGather -> full table.
  - aggregation: host pre-sorts each core's (self-loop-augmented) edges by
    (src window, dst block); windows are <=32768 table rows so dma_gather's
    int16 indices reach them (in_ap slides per window).  Compact 128-edge
    chunks are gathered in bulk, a bf16 one-hot of local dst is built with a
    single batched is_equal against an iota row, and a PE matmul accumulates
    each chunk into the dst block's PSUM tile; window contributions are
    copied/added into the SBUF u buffer.
  - u = u*dinv + b, then BatchNorm: per-core sums/sumsq (pad rows corrected
    exactly), one tiny AllReduce, stats finished on device, fused apply.
  - layer-1 h rows are pooled per graph directly in permuted slot order via
    uploaded one-hot matmuls, divided by counts, and run through the head.

The SPMD program is identical on every core (block/chunk counts maxed over
cores); all per-core variation is in uploaded index/scale tensors.
"""

import math

import ml_dtypes
import numpy as np

import concourse.bacc as bacc
import concourse.tile as tile
from concourse import mybir
from concourse.bass import AP
from concourse.bass_utils import run_bass_kernel_spmd

F32 = mybir.dt.float32
BF16 = mybir.dt.bfloat16
I16 = mybir.dt.int16
I32 = mybir.dt.int32
AX = mybir.AxisListType
OP = mybir.AluOpType
ACT = mybir.ActivationFunctionType
BF = ml_dtypes.bfloat16

EPS = 1e-5
NCORES = 8
H = 64
F_IN = 7
F_PAD = 8
TROW = 128           # table row width (bf16) = 256B
G_DEFAULT = 1024

WMAX = 32768
AGG_MODE = "full"         # dma_gather int16 index reach
SCALL = 96           # gather-call budget (columns of 128 edges)
OHB = 16             # one-hot build batch (columns)


def _ap(t, offset, dims):
    base = t[:] if not isinstance(t, AP) else t
    return AP(base.tensor, base.offset + offset, list(map(tuple, dims)))


# ----------------------------------------------------------------------------
# Host-side preprocessing (graph structure only; all model math is on device)
# ----------------------------------------------------------------------------

def _prep_branch(x, ei, batch, G, ncores):
    x = np.asarray(x, np.float32)
    N, Fin = x.shape
    assert Fin == F_IN
    src = np.asarray(ei[0], np.int64)
    dst = np.asarray(ei[1], np.int64)
    batch = np.asarray(batch, np.int64)

    counts = np.bincount(dst, minlength=N)
    deg = counts + 1
    dinv = (1.0 / np.sqrt(deg.astype(np.float64))).astype(np.float32)

    order = np.argsort(dst, kind="stable")
    src_sorted = src[order]
    rowptr = np.zeros(N + 1, np.int64)
    np.cumsum(counts, out=rowptr[1:])

    Gpc = G // ncores
    gb = np.searchsorted(batch, np.arange(0, G + 1, Gpc))

    cores = []
    for k in range(ncores):
        n0, n1 = int(gb[k]), int(gb[k + 1])
        NB = n1 - n0
        perm = np.argsort(-deg[n0:n1], kind="stable")
        node_order = n0 + perm
        cores.append(dict(n0=n0, NB=NB, node_order=node_order))

    nb = max(math.ceil(c["NB"] / 128) for c in cores)
    NBp = nb * 128
    Ppc = NBp + 128  # +1 zero block

    for k, c in enumerate(cores):
        no = np.full(NBp, -1, np.int64)
        no[: c["NB"]] = c["node_order"]
        c["node_order_p"] = no
        # per-edge flat arrays in slot space (incl self loop)
        slots = np.arange(c["NB"], dtype=np.int64)
        nodes = c["node_order"]
        lens = rowptr[nodes + 1] - rowptr[nodes]
        tot = int(lens.sum())
        cl = np.zeros(c["NB"] + 1, np.int64)
        np.cumsum(lens, out=cl[1:])
        r = np.arange(tot) - np.repeat(cl[:-1], lens)
        e_src = src_sorted[np.repeat(rowptr[nodes], lens) + r]
        e_dst = np.repeat(slots, lens)
        # append self loops
        c["e_src"] = np.concatenate([e_src, nodes])
        c["e_dst"] = np.concatenate([e_dst, slots])
        # per-core helper tensors
        dp = np.zeros(NBp, np.float32)
        dp[: c["NB"]] = dinv[c["node_order"]]
        c["dinvp"] = dp.reshape(nb, 128).T.copy()
        xp = np.zeros((NBp, F_PAD), np.float32)
        xp[: c["NB"], :F_IN] = x[c["node_order"]]
        c["xperm"] = xp
        oh = np.zeros((128, nb * Gpc), np.float32)
        rows = np.arange(c["NB"])
        g_local = batch[c["node_order"]] - k * Gpc
        oh[rows % 128, (rows // 128) * Gpc + g_local] = 1.0
        c["pool_oh"] = oh
        cg = np.bincount(batch, minlength=G)[k * Gpc : (k + 1) * Gpc]
        c["invcnt"] = (1.0 / np.maximum(cg.astype(np.float64), 1.0)).astype(
            np.float32
        )[:, None]
        c["padcnt"] = np.full((64, 1), NBp - c["NB"], np.float32)

    cfg = dict(N=N, nb=nb, NBp=NBp, Ppc=Ppc, Gpc=Gpc)
    return cfg, cores


def _build_schedule(cfgb, cores, rowmap, sec_off, Ppart, nwin, wrow, zrow_w):
    """Build the uniform chunk schedule + per-core idx/dst arrays for a branch."""
    nb = cfgb["nb"]
    ncores = len(cores)

    per_core_sorted = []
    counts = np.zeros((ncores, nwin, nb), np.int64)
    for k, c in enumerate(cores):
        srow = rowmap[c["e_src"]]
        w = srow // wrow
        blk = c["e_dst"] // 128
        key = np.lexsort((c["e_dst"], w))
        per_core_sorted.append((srow[key], c["e_dst"][key], w[key]))
        np.add.at(counts[k], (w, blk), 1)

    cnt = np.zeros((nwin, nb), np.int64)
    for w in range(nwin):
        for b in range(nb):
            cnt[w, b] = max(math.ceil(counts[k, w, b] / 128) for k in range(ncores))
    cnt[0] = np.maximum(cnt[0], 1)  # every block initialized in window 0

    sched = []  # (w, blk, ncols)
    for w in range(nwin):
        for b in range(nb):
            if cnt[w, b] > 0:
                sched.append((w, b, int(cnt[w, b])))
    totcols = int(sum(s[2] for s in sched))

    # calls: split at window boundaries and SCALL budget
    calls = []  # (w, col0, ncols)
    col = 0
    for w in range(nwin):
        wcols = int(sum(s[2] for s in sched if s[0] == w))
        done = 0
        while done < wcols:
            n = min(SCALL, wcols - done)
            calls.append((w, col, n))
            col += n
            done += n
    assert col == totcols

    # per-core arrays
    idx_all = []
    dst_all = []
    for k in range(ncores):
        srow, edst, ew = per_core_sorted[k]
        idx_un = np.zeros(totcols * 128, np.int16)
        dst_un = np.zeros(totcols * 128, np.int16)
        pos = 0  # edge pointer
        col0 = 0
        for (w, b, nc_) in sched:
            e = int(counts[k, w, b])
            sl = slice(col0 * 128, col0 * 128 + e)
            idx_un[sl] = (srow[pos : pos + e] - w * wrow).astype(np.int16)
            dst_un[sl] = (edst[pos : pos + e] - b * 128).astype(np.int16)
            pad = nc_ * 128 - e
            if pad:
                sl2 = slice(col0 * 128 + e, (col0 + nc_) * 128)
                idx_un[sl2] = zrow_w[w]
                dst_un[sl2] = 0
            pos += e
            col0 += nc_
        assert pos == len(srow)
        # wrap idx per call: item i -> [i%16, i//16]
        wrapped = np.zeros((16, totcols * 8), np.int16)
        for (w, c0, n) in calls:
            items = idx_un[c0 * 128 : (c0 + n) * 128]
            wrapped[:, c0 * 8 : (c0 + n) * 8] = items.reshape(n * 8, 16).T
        idx_all.append(np.tile(wrapped, (8, 1)))
        dst_all.append(
            np.ascontiguousarray(dst_un.reshape(totcols, 128).T).astype(BF)
        )

    # sanity: windows within int16
    assert wrow <= WMAX
    return dict(sched=sched, calls=calls, totcols=totcols), idx_all, dst_all


def _prep(inputs, G, ncores):
    cfg_c, cores_c = _prep_branch(
        inputs["x_c"], inputs["edge_index_c"], inputs["batch_c"], G, ncores
    )
    cfg_s, cores_s = _prep_branch(
        inputs["x_s"], inputs["edge_index_s"], inputs["batch_s"], G, ncores
    )
    Ppart = cfg_c["Ppc"] + cfg_s["Ppc"]
    Rtot = ncores * Ppart
    nwin = max(1, math.ceil(Rtot / WMAX))
    wrow = math.ceil(Rtot / nwin)

    # zero rows (last row of each section half is guaranteed zero)
    zrows = sorted(
        [k * Ppart + cfg_c["Ppc"] - 1 for k in range(ncores)]
        + [k * Ppart + Ppart - 1 for k in range(ncores)]
    )
    zrow_w = []
    for w in range(nwin):
        lo, hi = w * wrow, min((w + 1) * wrow, Rtot)
        z = [r for r in zrows if lo <= r < hi]
        assert z, f"no zero row in window {w}"
        zrow_w.append(z[0] - lo)

    # table row of each original node
    def rowmap_of(cfgb, cores, sec):
        N = cfgb["N"]
        rm = np.zeros(N, np.int64)
        for k, c in enumerate(cores):
            rm[c["node_order"]] = k * Ppart + sec + np.arange(c["NB"])
        return rm

    rm_c = rowmap_of(cfg_c, cores_c, 0)
    rm_s = rowmap_of(cfg_s, cores_s, cfg_c["Ppc"])

    sch_c, idx_c, dst_c = _build_schedule(
        cfg_c, cores_c, rm_c, 0, Ppart, nwin, wrow, zrow_w
    )
    sch_s, idx_s, dst_s = _build_schedule(
        cfg_s, cores_s, rm_s, cfg_c["Ppc"], Ppart, nwin, wrow, zrow_w
    )
    cfg_c.update(sch_c)
    cfg_s.update(sch_s)

    cfg = dict(
        c=cfg_c, s=cfg_s, Ppart=Ppart, Rtot=Rtot, nwin=nwin, wrow=wrow,
        G=G, Gpc=G // ncores,
    )

    w = {}
    for br in ("c", "s"):
        W0 = np.zeros((F_PAD, H), np.float32)
        W0[:F_IN] = np.asarray(inputs[f"W{br}0"], np.float32)
        w[f"W0_{br}"] = W0
        w[f"W1_{br}"] = np.asarray(inputs[f"W{br}1"], np.float32)
        for li in (0, 1):
            w[f"b{li}_{br}"] = np.asarray(inputs[f"b{br}{li}"], np.float32)[None, :]
            w[f"b{li}T_{br}"] = np.asarray(inputs[f"b{br}{li}"], np.float32)[:, None]
            w[f"g{li}_{br}"] = np.asarray(inputs[f"g{br}{li}"], np.float32)[:, None]
            w[f"beta{li}_{br}"] = np.asarray(
                inputs[f"beta{br}{li}"], np.float32
            )[:, None]
    w["Wf1"] = np.asarray(inputs["Wf1"], np.float32)
    w["bf1"] = np.asarray(inputs["bf1"], np.float32)[None, :]
    w["Wf2"] = np.asarray(inputs["Wf2"], np.float32)
    w["bf2"] = np.asarray(inputs["bf2"], np.float32)[None, :]

    in_maps = []
    for k in range(ncores):
        m = dict(w)
        for br, cores, idxs, dsts in (
            ("c", cores_c, idx_c, dst_c),
            ("s", cores_s, idx_s, dst_s),
        ):
            c = cores[k]
            m[f"xperm_{br}"] = c["xperm"]
            m[f"ell_{br}"] = idxs[k]
            m[f"dst_{br}"] = dsts[k]
            m[f"dinvp_{br}"] = c["dinvp"]
            m[f"pool_{br}"] = c["pool_oh"]
            m[f"invcnt_{br}"] = c["invcnt"]
            m[f"padcnt_{br}"] = c["padcnt"]
        in_maps.append(m)
    return cfg, in_maps


# ----------------------------------------------------------------------------
# Device program
# ----------------------------------------------------------------------------

def _build(cfg, stop=None):
    nc = bacc.Bacc(
        "TRN2", target_bir_lowering=False, debug=False, num_devices=NCORES
    )
    Gpc = cfg["Gpc"]
    Ppart = cfg["Ppart"]
    Rtot = cfg["Rtot"]
    wrow = cfg["wrow"]
    rg = [list(range(NCORES))]

    inp = {}
    for br in ("c", "s"):
        b = cfg[br]
        nb, totcols, NBp = b["nb"], b["totcols"], b["NBp"]
        inp[f"xperm_{br}"] = nc.dram_tensor(
            f"xperm_{br}", [NBp, F_PAD], F32, kind="ExternalInput"
        )
        inp[f"ell_{br}"] = nc.dram_tensor(
            f"ell_{br}", [128, totcols * 8], I16, kind="ExternalInput"
        )
        inp[f"dst_{br}"] = nc.dram_tensor(
            f"dst_{br}", [128, totcols], BF16, kind="ExternalInput"
        )
        inp[f"dinvp_{br}"] = nc.dram_tensor(
            f"dinvp_{br}", [128, nb], F32, kind="ExternalInput"
        )
        inp[f"pool_{br}"] = nc.dram_tensor(
            f"pool_{br}", [128, nb * Gpc], F32, kind="ExternalInput"
        )
        inp[f"invcnt_{br}"] = nc.dram_tensor(
            f"invcnt_{br}", [Gpc, 1], F32, kind="ExternalInput"
        )
        inp[f"padcnt_{br}"] = nc.dram_tensor(
            f"padcnt_{br}", [64, 1], F32, kind="ExternalInput"
        )
        inp[f"W0_{br}"] = nc.dram_tensor(
            f"W0_{br}", [F_PAD, H], F32, kind="ExternalInput"
        )
        inp[f"W1_{br}"] = nc.dram_tensor(f"W1_{br}", [H, H], F32, kind="ExternalInput")
        for li in (0, 1):
            for nm, shape in (
                (f"b{li}_{br}", [1, H]), (f"b{li}T_{br}", [H, 1]),
                (f"g{li}_{br}", [H, 1]), (f"beta{li}_{br}", [H, 1]),
            ):
                inp[nm] = nc.dram_tensor(nm, shape, F32, kind="ExternalInput")
    inp["Wf1"] = nc.dram_tensor("Wf1", [2 * H, H], F32, kind="ExternalInput")
    inp["bf1"] = nc.dram_tensor("bf1", [1, H], F32, kind="ExternalInput")
    inp["Wf2"] = nc.dram_tensor("Wf2", [H, 2], F32, kind="ExternalInput")
    inp["bf2"] = nc.dram_tensor("bf2", [1, 2], F32, kind="ExternalInput")

    out_t = nc.dram_tensor("out", [Gpc, 2], F32, kind="ExternalOutput")

    part = [nc.dram_tensor(f"part{li}", [Ppart, TROW], BF16) for li in (0, 1)]
    table = [
        nc.dram_tensor(f"table{li}", [Rtot, TROW], BF16, addr_space="Shared")
        for li in (0, 1)
    ]
    st_in = [nc.dram_tensor(f"st{li}_in", [4, H], F32) for li in (0, 1)]
    st_out = [
        nc.dram_tensor(f"st{li}_out", [4, H], F32, addr_space="Shared")
        for li in (0, 1)
    ]

    sec_off = {"c": 0, "s": cfg["c"]["Ppc"]}

    with tile.TileContext(nc, num_cores=NCORES) as tc:
        consts = tc.alloc_tile_pool(name="consts", bufs=1)
        wpool = tc.alloc_tile_pool(name="weights", bufs=1)
        upool = tc.alloc_tile_pool(name="ubuf", bufs=1)
        gpool = tc.alloc_tile_pool(name="gather", bufs=2)
        ohpool = tc.alloc_tile_pool(name="onehot", bufs=2)
        spool = tc.alloc_tile_pool(name="small", bufs=4)
        ppool = tc.alloc_tile_pool(name="psum", bufs=1, space="PSUM")
        xpool = tc.alloc_tile_pool(name="xstage", bufs=3)

        from concourse.masks import make_identity

        ident = consts.tile([128, 128], F32)
        make_identity(nc, ident[:])
        ones_col = consts.tile([128, 1], F32)
        nc.gpsimd.memset(ones_col[:], 1.0)
        ones_row = consts.tile([1, 128], F32)
        nc.gpsimd.memset(ones_row[:], 1.0)
        eps_t = consts.tile([H, 1], F32)
        nc.gpsimd.memset(eps_t[:], EPS)
        zero_big = consts.tile([128, 1024], BF16)
        nc.gpsimd.memset(zero_big[:], 0.0)
        iota_i = consts.tile([128, 128], I32)
        nc.gpsimd.iota(iota_i[:], pattern=[[1, 128]], base=0, channel_multiplier=0)
        iota_bf = consts.tile([128, 128], BF16)
        nc.vector.tensor_copy(out=iota_bf[:], in_=iota_i[:])

        def load_w(name, shape):
            t = wpool.tile(list(shape), F32, tag=name, name=f"w_{name}")
            nc.sync.dma_start(out=t[:], in_=inp[name].ap())
            return t

        def replicate_row(row_ap, width, tag):
            ps = ppool.tile([128, width], F32, tag="ps_u", bufs=3)
            nc.tensor.matmul(
                out=ps[:], lhsT=ones_row[:], rhs=row_ap, start=True, stop=True
            )
            t = wpool.tile([128, width], F32, tag=tag, name=f"rep_{tag}")
            nc.vector.tensor_copy(out=t[:], in_=ps[:])
            return t

        Wt = {}
        for br in ("c", "s"):
            Wt[br, 0] = load_w(f"W0_{br}", (F_PAD, H))
            Wt[br, 1] = load_w(f"W1_{br}", (H, H))
        Wf1 = load_w("Wf1", (2 * H, H))
        Wf2 = load_w("Wf2", (H, 2))
        bT = {}
        gam = {}
        bet = {}
        brep = {}
        for br in ("c", "s"):
            for li in (0, 1):
                bT[br, li] = load_w(f"b{li}T_{br}", (H, 1))
                gam[br, li] = load_w(f"g{li}_{br}", (H, 1))
                bet[br, li] = load_w(f"beta{li}_{br}", (H, 1))
                brow = spool.tile([1, H], F32, tag="brow")
                nc.sync.dma_start(out=brow[:], in_=inp[f"b{li}_{br}"].ap())
                brep[br, li] = replicate_row(brow[:], H, f"brep_{br}{li}")
        bf1row = spool.tile([1, H], F32, tag="brow")
        nc.sync.dma_start(out=bf1row[:], in_=inp["bf1"].ap())
        bf1rep = replicate_row(bf1row[:], H, "bf1rep")
        bf2row = spool.tile([1, 2], F32, tag="brow2")
        nc.sync.dma_start(out=bf2row[:], in_=inp["bf2"].ap())
        bf2rep = replicate_row(bf2row[:], 2, "bf2rep")

        dinvp_t = {}
        padcnt_t = {}
        for br in ("c", "s"):
            nb = cfg[br]["nb"]
            dinvp_t[br] = wpool.tile(
                [128, nb], F32, name=f"dinvp_t_{br}", tag=f"dinvp_{br}"
            )
            nc.sync.dma_start(out=dinvp_t[br][:], in_=inp[f"dinvp_{br}"].ap())
            padcnt_t[br] = wpool.tile(
                [64, 1], F32, name=f"padcnt_t_{br}", tag=f"padcnt_{br}"
            )
            nc.sync.dma_start(out=padcnt_t[br][:], in_=inp[f"padcnt_{br}"].ap())

        # bulk-zero both parts (covers high halves, pad rows, zero blocks)
        for li in (0, 1):
            total = Ppart * TROW
            off = 0
            while off < total:
                n = min(128 * 1024, total - off)
                ncols = n // 128
                nc.sync.dma_start(
                    out=_ap(part[li].ap(), off, [(ncols, 128), (1, ncols)]),
                    in_=zero_big[:, :ncols],
                )
                off += n

        u_t = {}
        acc2_t = {}

        # ------------------------------------------------------------------
        # table section build: rows <- (dinv * src_rows) @ W  (bf16, cols 0:64)
        # ------------------------------------------------------------------
        def build_table(br, li):
            b = cfg[br]
            nb = b["nb"]
            for t in range(nb):
                if li == 0:
                    xt = xpool.tile([128, F_PAD], F32, tag="xt")
                    nc.sync.dma_start(
                        out=xt[:],
                        in_=_ap(inp[f"xperm_{br}"].ap(), t * 128 * F_PAD,
                                [(F_PAD, 128), (1, F_PAD)]),
                    )
                    fin = F_PAD
                else:
                    u = u_t[br]
                    xt = xpool.tile([128, H], F32, tag="xt1")
                    nc.vector.tensor_tensor(
                        out=xt[:], in0=u[:, t * H : (t + 1) * H],
                        in1=_ap(dinvp_t[br], t, [(cfg[br]["nb"], 128), (0, H)]),
                        op=OP.mult,
                    )
                    fin = H
                if li == 0:
                    nc.vector.tensor_tensor(
                        out=xt[:], in0=xt[:],
                        in1=_ap(dinvp_t[br], t, [(nb, 128), (0, F_PAD)]),
                        op=OP.mult,
                    )
                zT_ps = ppool.tile([fin, 128], F32, tag="ps_t", bufs=3)
                nc.tensor.transpose(out=zT_ps[:], in_=xt[:], identity=ident[:])
                zT = xpool.tile([fin, 128], F32, tag=f"zT{fin}")
                nc.vector.tensor_copy(out=zT[:], in_=zT_ps[:])
                r_ps = ppool.tile([128, H], F32, tag="ps_u", bufs=3)
                nc.tensor.matmul(
                    out=r_ps[:], lhsT=zT[:], rhs=Wt[br, li][:], start=True, stop=True
                )
                stage = xpool.tile([128, H], BF16, tag="stage")
                nc.vector.tensor_copy(out=stage[:], in_=r_ps[:])
                nc.sync.dma_start(
                    out=_ap(part[li].ap(), (sec_off[br] + t * 128) * TROW,
                            [(TROW, 128), (1, H)]),
                    in_=stage[:],
                )

        # ------------------------------------------------------------------
        # aggregation: u[:, blk] = sum over windows of one-hot matmuls
        # ------------------------------------------------------------------
        def aggregate(br, li):
            b = cfg[br]
            nb = b["nb"]
            u = u_t.get(br)
            if u is None:
                u = upool.tile([128, nb * H], F32, tag=f"u_{br}", name=f"u_{br}")
                u_t[br] = u
            # expand schedule into per-column (w, blk, first, last)
            colinfo = []
            for (w, blk, cnt) in b["sched"]:
                for i in range(cnt):
                    colinfo.append((w, blk, i == 0, i == cnt - 1))
            ci = 0
            for (w, col0, ncols) in b["calls"]:
                it = gpool.tile([128, SCALL * 8], I16, tag="it")
                nc.sync.dma_start(
                    out=it[:, : ncols * 8],
                    in_=inp[f"ell_{br}"].ap()[:, col0 * 8 : (col0 + ncols) * 8],
                )
                dstc = gpool.tile([128, SCALL], BF16, tag="dstc")
                nc.sync.dma_start(
                    out=dstc[:, :ncols],
                    in_=inp[f"dst_{br}"].ap()[:, col0 : col0 + ncols],
                )
                gbuf = gpool.tile([128, SCALL * TROW], BF16, tag="gbuf")
                num = ncols * 128
                nc.gpsimd.dma_gather(
                    out_ap=_ap(gbuf, 0, [(SCALL * TROW, 128), (TROW, ncols), (1, TROW)]),
                    in_ap=table[li].ap()[w * wrow :, :],
                    idxs_ap=it[:, : ncols * 8],
                    num_idxs=num,
                    num_idxs_reg=num,
                    elem_size=TROW,
                    single_packet=False,
                )
                if AGG_MODE == "gather":
                    ci += ncols
                    continue
                for b0 in range(0, ncols, OHB):
                    bn = min(OHB, ncols - b0)
                    oh = ohpool.tile([128, OHB * 128], BF16, tag="oh")
                    nc.vector.tensor_tensor(
                        out=_ap(oh, 0, [(OHB * 128, 128), (128, bn), (1, 128)]),
                        in0=_ap(dstc, b0, [(SCALL, 128), (1, bn), (0, 128)]),
                        in1=_ap(iota_bf, 0, [(128, 128), (0, bn), (1, 128)]),
                        op=OP.is_equal,
                    )
                    if AGG_MODE == "onehot":
                        ci += bn
                        continue
                    for j in range(bn):
                        cw, cblk, cfirst, clast = colinfo[ci]
                        ci += 1
                        if cfirst:
                            z_ps = ppool.tile(
                                [128, H], F32, tag="ps_z", bufs=2,
                                name=f"zps_{br}{li}_{ci}",
                            )
                            aggregate.cur[br] = z_ps
                        z_ps = aggregate.cur[br]
                        nc.tensor.matmul(
                            out=z_ps[:],
                            lhsT=oh[:, (j + b0 - b0) * 128 : (j + 1) * 128]
                            if False
                            else oh[:, j * 128 : (j + 1) * 128],
                            rhs=_ap(gbuf, (b0 + j) * TROW,
                                    [(SCALL * TROW, 128), (1, H)]),
                            start=cfirst,
                            stop=clast,
                        )
                        if clast:
                            ucol = u[:, cblk * H : (cblk + 1) * H]
                            if cw == 0:
                                nc.vector.tensor_copy(out=ucol, in_=z_ps[:])
                            else:
                                nc.vector.tensor_tensor(
                                    out=ucol, in0=ucol, in1=z_ps[:], op=OP.add
                                )
            assert ci == len(colinfo)
            if AGG_MODE in ("gather", "onehot", "matmul_nou"):
                nc.gpsimd.memset(u[:], 0.0)
            # u = u * dinvp + b
            full = _ap(u, 0, [(nb * H, 128), (H, nb), (1, H)])
            nc.vector.tensor_tensor(
                out=full, in0=full,
                in1=_ap(dinvp_t[br], 0, [(nb, 128), (1, nb), (0, H)]), op=OP.mult,
            )
            nc.vector.tensor_tensor(
                out=full, in0=full,
                in1=_ap(brep[br, li], 0, [(H, 128), (0, nb), (1, H)]), op=OP.add,
            )

        aggregate.cur = {}

        def layer_stats(br, li, st_in_t, row):
            b = cfg[br]
            nb = b["nb"]
            u = u_t[br]
            acc2 = spool.tile([128, H], F32, tag=f"acc2_{br}", name=f"acc2_{br}{li}")
            acc2_t[br] = acc2
            sq = spool.tile([128, H], F32, tag="sq")
            nc.scalar.activation(
                out=sq[:], in_=u[:, 0:H], func=ACT.Square
            )
            nc.vector.tensor_copy(out=acc2[:], in_=sq[:])
            for t in range(1, nb):
                sq = spool.tile([128, H], F32, tag="sq")
                nc.scalar.activation(
                    out=sq[:], in_=u[:, t * H : (t + 1) * H], func=ACT.Square
                )
                nc.vector.tensor_tensor(
                    out=acc2[:], in0=acc2[:], in1=sq[:], op=OP.add
                )
            rsum = spool.tile([128, H], F32, tag="rsum")
            nc.vector.tensor_reduce(
                out=rsum[:], in_=_ap(u, 0, [(nb * H, 128), (1, H), (H, nb)]),
                axis=AX.X, op=OP.add,
            )
            su_ps = ppool.tile([H, 1], F32, tag="ps_t", bufs=3)
            nc.tensor.matmul(
                out=su_ps[:], lhsT=rsum[:], rhs=ones_col[:], start=True, stop=True
            )
            s2_ps = ppool.tile([H, 1], F32, tag="ps_t", bufs=3)
            nc.tensor.matmul(
                out=s2_ps[:], lhsT=acc2[:], rhs=ones_col[:], start=True, stop=True
            )
            corr = spool.tile([H, 1], F32, tag="corr")
            nc.vector.tensor_tensor(
                out=corr[:], in0=padcnt_t[br][:], in1=bT[br, li][:], op=OP.mult
            )
            su = spool.tile([H, 1], F32, tag="su")
            nc.vector.tensor_tensor(out=su[:], in0=su_ps[:], in1=corr[:], op=OP.subtract)
            bsq = spool.tile([H, 1], F32, tag="bsq")
            nc.vector.tensor_tensor(
                out=bsq[:], in0=bT[br, li][:], in1=bT[br, li][:], op=OP.mult
            )
            nc.vector.tensor_tensor(
                out=bsq[:], in0=bsq[:], in1=padcnt_t[br][:], op=OP.mult
            )
            s2 = spool.tile([H, 1], F32, tag="s2")
            nc.vector.tensor_tensor(out=s2[:], in0=s2_ps[:], in1=bsq[:], op=OP.subtract)
            nc.sync.dma_start(out=st_in_t.ap()[row : row + 1, :], in_=su[:])
            nc.sync.dma_start(out=st_in_t.ap()[row + 1 : row + 2, :], in_=s2[:])

        def bn_finish(br, li, st_out_t, row, Ntotal):
            sts = spool.tile([H, 2], F32, tag="sts")
            nc.sync.dma_start(
                out=sts[:], in_=_ap(st_out_t.ap(), row * H, [(1, H), (H, 2)])
            )
            mu = spool.tile([H, 1], F32, tag="mu")
            nc.vector.tensor_scalar_mul(out=mu[:], in0=sts[:, 0:1], scalar1=1.0 / Ntotal)
            ex2 = spool.tile([H, 1], F32, tag="ex2")
            nc.vector.tensor_scalar_mul(out=ex2[:], in0=sts[:, 1:2], scalar1=1.0 / Ntotal)
            musq = spool.tile([H, 1], F32, tag="musq")
            nc.vector.tensor_tensor(out=musq[:], in0=mu[:], in1=mu[:], op=OP.mult)
            var = spool.tile([H, 1], F32, tag="var")
            nc.vector.tensor_tensor(out=var[:], in0=ex2[:], in1=musq[:], op=OP.subtract)
            std = spool.tile([H, 1], F32, tag="std")
            nc.scalar.activation(out=std[:], in_=var[:], func=ACT.Sqrt, bias=eps_t[:])
            istd = spool.tile([H, 1], F32, tag="istd")
            nc.vector.reciprocal(out=istd[:], in_=std[:])
            sc = spool.tile([H, 1], F32, tag="sc")
            nc.vector.tensor_tensor(out=sc[:], in0=gam[br, li][:], in1=istd[:], op=OP.mult)
            sh = spool.tile([H, 1], F32, tag="sh")
            nc.vector.tensor_tensor(out=sh[:], in0=mu[:], in1=sc[:], op=OP.mult)
            nc.vector.tensor_tensor(
                out=sh[:], in0=bet[br, li][:], in1=sh[:], op=OP.subtract
            )
            reps = []
            for vec, tag in ((sc, "screp"), (sh, "shrep")):
                vr_ps = ppool.tile([1, H], F32, tag="ps_t", bufs=3)
                nc.tensor.transpose(out=vr_ps[:], in_=vec[:], identity=ident[:H, :H])
                vr = spool.tile([1, H], F32, tag="vrow")
                nc.vector.tensor_copy(out=vr[:], in_=vr_ps[:])
                reps.append(replicate_row(vr[:], H, f"{tag}_{br}{li}"))
            return reps

        def bn_apply(br, screp, shrep):
            b = cfg[br]
            nb = b["nb"]
            u = u_t[br]
            full = _ap(u, 0, [(nb * H, 128), (H, nb), (1, H)])
            nc.vector.tensor_tensor(
                out=full, in0=full,
                in1=_ap(screp, 0, [(H, 128), (0, nb), (1, H)]), op=OP.mult,
            )
            nc.vector.tensor_tensor(
                out=full, in0=full,
                in1=_ap(shrep, 0, [(H, 128), (0, nb), (1, H)]), op=OP.add,
            )
            flat = u[:, : nb * H]
            nc.scalar.activation(out=flat, in_=flat, func=ACT.Relu)

        # =============================== flow ===============================
        def early_out(src_ap):
            o = spool.tile([Gpc, 2], F32, tag="o_sb")
            nc.vector.tensor_copy(out=o[:], in_=src_ap)
            nc.sync.dma_start(out=out_t.ap(), in_=o[:])

        for br in ("c", "s"):
            build_table(br, 0)
        nc.gpsimd.collective_compute(
            "AllGather", OP.bypass, replica_groups=rg,
            ins=[part[0].ap()], outs=[table[0].ap()],
        )
        if stop == "t0":
            tt = spool.tile([Gpc, 2], BF16, tag="tt")
            nc.sync.dma_start(out=tt[:], in_=_ap(table[0].ap(), 0, [(TROW, Gpc), (1, 2)]))
            early_out(tt[:])
        if stop is None or stop > "t0":
            for br in ("c", "s"):
                aggregate(br, 0)
            if stop == "t1":
                early_out(u_t["c"][:Gpc, :2])
        if stop is None or stop > "t1":
            layer_stats("c", 0, st_in[0], 0)
            layer_stats("s", 0, st_in[0], 2)
            nc.gpsimd.collective_compute(
                "AllReduce", OP.add, replica_groups=rg,
                ins=[st_in[0].ap()], outs=[st_out[0].ap()],
            )
            for br, Ntot in (("c", cfg["c"]["N"]), ("s", cfg["s"]["N"])):
                screp, shrep = bn_finish(br, 0, st_out[0], 0 if br == "c" else 2, Ntot)
                bn_apply(br, screp, shrep)
                build_table(br, 1)
            if stop == "t2":
                early_out(u_t["c"][:Gpc, :2])
        if stop is None or stop > "t2":
            nc.gpsimd.collective_compute(
                "AllGather", OP.bypass, replica_groups=rg,
                ins=[part[1].ap()], outs=[table[1].ap()],
            )
            for br in ("c", "s"):
                aggregate(br, 1)
            if stop == "t3":
                early_out(u_t["c"][:Gpc, :2])
        if stop is None or stop > "t3":
            layer_stats("c", 1, st_in[1], 0)
            layer_stats("s", 1, st_in[1], 2)
            nc.gpsimd.collective_compute(
                "AllReduce", OP.add, replica_groups=rg,
                ins=[st_in[1].ap()], outs=[st_out[1].ap()],
            )
        hcat = spool.tile([Gpc, 2 * H], F32, tag="hcat")
        tail = [] if stop is not None else [
            ("c", cfg["c"]["N"]), ("s", cfg["s"]["N"])]
        for br, Ntot in tail:
            screp, shrep = bn_finish(br, 1, st_out[1], 0 if br == "c" else 2, Ntot)
            bn_apply(br, screp, shrep)
            b = cfg[br]
            nb = b["nb"]
            u = u_t[br]
            pool_ps = ppool.tile(
                [Gpc, H], F32, tag="ps_u", bufs=3, name=f"pool_ps_{br}"
            )
            for t in range(nb):
                poh = gpool.tile([128, Gpc], F32, tag="poh")
                nc.sync.dma_start(
                    out=poh[:],
                    in_=inp[f"pool_{br}"].ap()[:, t * Gpc : (t + 1) * Gpc],
                )
                nc.tensor.matmul(
                    out=pool_ps[:], lhsT=poh[:], rhs=u[:, t * H : (t + 1) * H],
                    start=(t == 0), stop=(t == nb - 1),
                )
            ic = spool.tile([Gpc, 1], F32, tag="ic")
            nc.sync.dma_start(out=ic[:], in_=inp[f"invcnt_{br}"].ap())
            col0 = 0 if br == "c" else H
            nc.vector.tensor_tensor(
                out=hcat[:, col0 : col0 + H], in0=pool_ps[:],
                in1=_ap(ic, 0, [(1, Gpc), (0, H)]), op=OP.mult,
            )

        if stop is not None:
            hcT_ps = None
        else:
            hcT_ps = ppool.tile([2 * H, Gpc], F32, tag="ps_t", bufs=3)
            nc.tensor.transpose(out=hcT_ps[:], in_=hcat[:], identity=ident[:Gpc, :Gpc])
        if stop is None:
            hcT = spool.tile([2 * H, Gpc], F32, tag="hcT")
            nc.vector.tensor_copy(out=hcT[:], in_=hcT_ps[:])
            f_ps = ppool.tile([Gpc, H], F32, tag="ps_u", bufs=3)
            nc.tensor.matmul(out=f_ps[:], lhsT=hcT[:], rhs=Wf1[:], start=True, stop=True)
            f_sb = spool.tile([Gpc, H], F32, tag="f_sb")
            nc.vector.tensor_tensor(out=f_sb[:], in0=f_ps[:], in1=bf1rep[:Gpc, :], op=OP.add)
            nc.scalar.activation(out=f_sb[:], in_=f_sb[:], func=ACT.Relu)
            fT_ps = ppool.tile([H, Gpc], F32, tag="ps_t", bufs=3)
            nc.tensor.transpose(out=fT_ps[:], in_=f_sb[:], identity=ident[:Gpc, :Gpc])
            fT = spool.tile([H, Gpc], F32, tag="fT")
            nc.vector.tensor_copy(out=fT[:], in_=fT_ps[:])
            o_ps = ppool.tile([Gpc, 2], F32, tag="ps_u", bufs=3)
            nc.tensor.matmul(out=o_ps[:], lhsT=fT[:], rhs=Wf2[:], start=True, stop=True)
            o_sb = spool.tile([Gpc, 2], F32, tag="o_sb")
            nc.vector.tensor_tensor(out=o_sb[:], in0=o_ps[:], in1=bf2rep[:Gpc, :], op=OP.add)
            nc.sync.dma_start(out=out_t.ap(), in_=o_sb[:])

        for p in (xpool, ppool, spool, ohpool, gpool, upool, wpool, consts):
            p.release()

    nc.compile()
    return nc


def kernel(_G=G_DEFAULT, _trace=False, _return_results=False, _stop=None, **inputs):
    cfg, in_maps = _prep(inputs, _G, NCORES)
    nc = _build(cfg, stop=_stop)
    res = run_bass_kernel_spmd(
        nc, in_maps, core_ids=list(range(NCORES)), trace=_trace
    )
    out = np.concatenate([res.results[k]["out"] for k in range(NCORES)], axis=0)
    if _return_results:
        return out, res
    return out


# revision 18
# speedup vs baseline: 3.1646x; 3.1646x over previous
"""DualGNN (2x [GCN->BN->ReLU]x2 -> mean-pool -> MLP head) on 8 trn2 NeuronCores.

Strategy
-------
Graphs are data-parallel: core k owns graphs [k*Gpc, (k+1)*Gpc) (batch is
sorted -> contiguous node ranges).  Edges are uniformly random over all nodes,
so each layer gathers rows of a replicated node-feature table.

Linear layers commute with aggregation, so W is applied per *node* before
gathering: the gather tables hold (dinv * h) @ W rows (bf16, padded to
256B rows for dma_gather).  Per core, per layer:
  - table section build: per-128-slot tile: scale by dinv, PE-transpose,
    matmul W, cast bf16, write to this core's part; AllGather -> full table.
  - aggregation: host pre-sorts each core's (self-loop-augmented) edges by
    (src window, dst block); windows are <=32768 table rows so dma_gather's
    int16 indices reach them (in_ap slides per window).  Compact 128-edge
    chunks are gathered in bulk, a bf16 one-hot of local dst is built with a
    single batched is_equal against an iota row, and a PE matmul accumulates
    each chunk into the dst block's PSUM tile; window contributions are
    copied/added into the SBUF u buffer.
  - u = u*dinv + b, then BatchNorm: per-core sums/sumsq (pad rows corrected
    exactly), one tiny AllReduce, stats finished on device, fused apply.
  - layer-1 h rows are pooled per graph directly in permuted slot order via
    uploaded one-hot matmuls, divided by counts, and run through the head.

The SPMD program is identical on every core (block/chunk counts maxed over
cores); all per-core variation is in uploaded index/scale tensors.
"""

import math

import ml_dtypes
import numpy as np

import concourse.bacc as bacc
import concourse.tile as tile
from concourse import mybir
from concourse.bass import AP
from concourse.bass_utils import run_bass_kernel_spmd

F32 = mybir.dt.float32
BF16 = mybir.dt.bfloat16
I16 = mybir.dt.int16
I32 = mybir.dt.int32
AX = mybir.AxisListType
OP = mybir.AluOpType
ACT = mybir.ActivationFunctionType
BF = ml_dtypes.bfloat16

EPS = 1e-5
NCORES = 8
H = 64
F_IN = 7
F_PAD = 8
TROW = 128           # table row width (bf16) = 256B
G_DEFAULT = 1024

WMAX = 32768
AGG_MODE = "full"         # dma_gather int16 index reach
SCALL = 96           # gather-call budget (columns of 128 edges)
OHB = 16             # one-hot build batch (columns)


def _ap(t, offset, dims):
    base = t[:] if not isinstance(t, AP) else t
    return AP(base.tensor, base.offset + offset, list(map(tuple, dims)))


# ----------------------------------------------------------------------------
# Host-side preprocessing (graph structure only; all model math is on device)
# ----------------------------------------------------------------------------

def _prep_branch(x, ei, batch, G, ncores):
    x = np.asarray(x, np.float32)
    N, Fin = x.shape
    assert Fin == F_IN
    src = np.asarray(ei[0], np.int64)
    dst = np.asarray(ei[1], np.int64)
    batch = np.asarray(batch, np.int64)

    counts = np.bincount(dst, minlength=N)
    deg = counts + 1
    dinv = (1.0 / np.sqrt(deg.astype(np.float64))).astype(np.float32)

    order = np.argsort(dst, kind="stable")
    src_sorted = src[order]
    rowptr = np.zeros(N + 1, np.int64)
    np.cumsum(counts, out=rowptr[1:])

    Gpc = G // ncores
    gb = np.searchsorted(batch, np.arange(0, G + 1, Gpc))

    cores = []
    for k in range(ncores):
        n0, n1 = int(gb[k]), int(gb[k + 1])
        NB = n1 - n0
        perm = np.argsort(-deg[n0:n1], kind="stable")
        node_order = n0 + perm
        cores.append(dict(n0=n0, NB=NB, node_order=node_order))

    nb = max(math.ceil(c["NB"] / 128) for c in cores)
    NBp = nb * 128
    Ppc = NBp + 128  # +1 zero block

    for k, c in enumerate(cores):
        no = np.full(NBp, -1, np.int64)
        no[: c["NB"]] = c["node_order"]
        c["node_order_p"] = no
        # per-edge flat arrays in slot space (incl self loop)
        slots = np.arange(c["NB"], dtype=np.int64)
        nodes = c["node_order"]
        lens = rowptr[nodes + 1] - rowptr[nodes]
        tot = int(lens.sum())
        cl = np.zeros(c["NB"] + 1, np.int64)
        np.cumsum(lens, out=cl[1:])
        r = np.arange(tot) - np.repeat(cl[:-1], lens)
        e_src = src_sorted[np.repeat(rowptr[nodes], lens) + r]
        e_dst = np.repeat(slots, lens)
        # append self loops
        c["e_src"] = np.concatenate([e_src, nodes])
        c["e_dst"] = np.concatenate([e_dst, slots])
        # per-core helper tensors
        dp = np.zeros(NBp, np.float32)
        dp[: c["NB"]] = dinv[c["node_order"]]
        c["dinvp"] = dp.reshape(nb, 128).T.copy()
        xp = np.zeros((NBp, F_PAD), np.float32)
        xp[: c["NB"], :F_IN] = x[c["node_order"]]
        c["xperm"] = xp
        oh = np.zeros((128, nb * Gpc), np.float32)
        rows = np.arange(c["NB"])
        g_local = batch[c["node_order"]] - k * Gpc
        oh[rows % 128, (rows // 128) * Gpc + g_local] = 1.0
        c["pool_oh"] = oh
        cg = np.bincount(batch, minlength=G)[k * Gpc : (k + 1) * Gpc]
        c["invcnt"] = (1.0 / np.maximum(cg.astype(np.float64), 1.0)).astype(
            np.float32
        )[:, None]
        c["padcnt"] = np.full((64, 1), NBp - c["NB"], np.float32)

    cfg = dict(N=N, nb=nb, NBp=NBp, Ppc=Ppc, Gpc=Gpc)
    return cfg, cores


def _build_schedule(cfgb, cores, rowmap, sec_off, Ppart, nwin, wrow, zrow_w):
    """Build the uniform chunk schedule + per-core idx/dst arrays for a branch."""
    nb = cfgb["nb"]
    ncores = len(cores)

    per_core_sorted = []
    counts = np.zeros((ncores, nwin, nb), np.int64)
    for k, c in enumerate(cores):
        srow = rowmap[c["e_src"]]
        w = srow // wrow
        blk = c["e_dst"] // 128
        key = np.lexsort((c["e_dst"], w))
        per_core_sorted.append((srow[key], c["e_dst"][key], w[key]))
        np.add.at(counts[k], (w, blk), 1)

    cnt = np.zeros((nwin, nb), np.int64)
    for w in range(nwin):
        for b in range(nb):
            cnt[w, b] = max(math.ceil(counts[k, w, b] / 128) for k in range(ncores))
    cnt[0] = np.maximum(cnt[0], 1)  # every block initialized in window 0

    sched = []  # (w, blk, ncols)
    for w in range(nwin):
        for b in range(nb):
            if cnt[w, b] > 0:
                sched.append((w, b, int(cnt[w, b])))
    totcols = int(sum(s[2] for s in sched))

    # calls: split at window boundaries and SCALL budget
    calls = []  # (w, col0, ncols)
    col = 0
    for w in range(nwin):
        wcols = int(sum(s[2] for s in sched if s[0] == w))
        done = 0
        while done < wcols:
            n = min(SCALL, wcols - done)
            calls.append((w, col, n))
            col += n
            done += n
    assert col == totcols

    # per-core arrays
    idx_all = []
    dst_all = []
    for k in range(ncores):
        srow, edst, ew = per_core_sorted[k]
        idx_un = np.zeros(totcols * 128, np.int16)
        dst_un = np.zeros(totcols * 128, np.int16)
        pos = 0  # edge pointer
        col0 = 0
        for (w, b, nc_) in sched:
            e = int(counts[k, w, b])
            sl = slice(col0 * 128, col0 * 128 + e)
            idx_un[sl] = (srow[pos : pos + e] - w * wrow).astype(np.int16)
            dst_un[sl] = (edst[pos : pos + e] - b * 128).astype(np.int16)
            pad = nc_ * 128 - e
            if pad:
                sl2 = slice(col0 * 128 + e, (col0 + nc_) * 128)
                idx_un[sl2] = zrow_w[w]
                dst_un[sl2] = 0
            pos += e
            col0 += nc_
        assert pos == len(srow)
        # wrap idx per call: item i -> [i%16, i//16]
        wrapped = np.zeros((16, totcols * 8), np.int16)
        for (w, c0, n) in calls:
            items = idx_un[c0 * 128 : (c0 + n) * 128]
            wrapped[:, c0 * 8 : (c0 + n) * 8] = items.reshape(n * 8, 16).T
        idx_all.append(np.tile(wrapped, (8, 1)))
        dst_all.append(
            np.ascontiguousarray(dst_un.reshape(totcols, 128).T).astype(BF)
        )

    # sanity: windows within int16
    assert wrow <= WMAX
    return dict(sched=sched, calls=calls, totcols=totcols), idx_all, dst_all


def _prep(inputs, G, ncores):
    cfg_c, cores_c = _prep_branch(
        inputs["x_c"], inputs["edge_index_c"], inputs["batch_c"], G, ncores
    )
    cfg_s, cores_s = _prep_branch(
        inputs["x_s"], inputs["edge_index_s"], inputs["batch_s"], G, ncores
    )
    Ppart = cfg_c["Ppc"] + cfg_s["Ppc"]
    Rtot = ncores * Ppart
    nwin = max(1, math.ceil(Rtot / WMAX))
    wrow = math.ceil(Rtot / nwin)

    # zero rows (last row of each section half is guaranteed zero)
    zrows = sorted(
        [k * Ppart + cfg_c["Ppc"] - 1 for k in range(ncores)]
        + [k * Ppart + Ppart - 1 for k in range(ncores)]
    )
    zrow_w = []
    for w in range(nwin):
        lo, hi = w * wrow, min((w + 1) * wrow, Rtot)
        z = [r for r in zrows if lo <= r < hi]
        assert z, f"no zero row in window {w}"
        zrow_w.append(z[0] - lo)

    # table row of each original node
    def rowmap_of(cfgb, cores, sec):
        N = cfgb["N"]
        rm = np.zeros(N, np.int64)
        for k, c in enumerate(cores):
            rm[c["node_order"]] = k * Ppart + sec + np.arange(c["NB"])
        return rm

    rm_c = rowmap_of(cfg_c, cores_c, 0)
    rm_s = rowmap_of(cfg_s, cores_s, cfg_c["Ppc"])

    sch_c, idx_c, dst_c = _build_schedule(
        cfg_c, cores_c, rm_c, 0, Ppart, nwin, wrow, zrow_w
    )
    sch_s, idx_s, dst_s = _build_schedule(
        cfg_s, cores_s, rm_s, cfg_c["Ppc"], Ppart, nwin, wrow, zrow_w
    )
    cfg_c.update(sch_c)
    cfg_s.update(sch_s)

    cfg = dict(
        c=cfg_c, s=cfg_s, Ppart=Ppart, Rtot=Rtot, nwin=nwin, wrow=wrow,
        G=G, Gpc=G // ncores,
    )

    w = {}
    for br in ("c", "s"):
        W0 = np.zeros((F_PAD, H), np.float32)
        W0[:F_IN] = np.asarray(inputs[f"W{br}0"], np.float32)
        w[f"W0_{br}"] = W0
        w[f"W1_{br}"] = np.asarray(inputs[f"W{br}1"], np.float32)
        for li in (0, 1):
            w[f"b{li}_{br}"] = np.asarray(inputs[f"b{br}{li}"], np.float32)[None, :]
            w[f"b{li}T_{br}"] = np.asarray(inputs[f"b{br}{li}"], np.float32)[:, None]
            w[f"g{li}_{br}"] = np.asarray(inputs[f"g{br}{li}"], np.float32)[:, None]
            w[f"beta{li}_{br}"] = np.asarray(
                inputs[f"beta{br}{li}"], np.float32
            )[:, None]
    w["Wf1"] = np.asarray(inputs["Wf1"], np.float32)
    w["bf1"] = np.asarray(inputs["bf1"], np.float32)[None, :]
    w["Wf2"] = np.asarray(inputs["Wf2"], np.float32)
    w["bf2"] = np.asarray(inputs["bf2"], np.float32)[None, :]

    in_maps = []
    for k in range(ncores):
        m = dict(w)
        for br, cores, idxs, dsts in (
            ("c", cores_c, idx_c, dst_c),
            ("s", cores_s, idx_s, dst_s),
        ):
            c = cores[k]
            m[f"xperm_{br}"] = c["xperm"]
            m[f"ell_{br}"] = idxs[k]
            m[f"dst_{br}"] = dsts[k]
            m[f"dinvp_{br}"] = c["dinvp"]
            m[f"pool_{br}"] = c["pool_oh"]
            m[f"invcnt_{br}"] = c["invcnt"]
            m[f"padcnt_{br}"] = c["padcnt"]
        in_maps.append(m)
    return cfg, in_maps


# ----------------------------------------------------------------------------
# Device program
# ----------------------------------------------------------------------------

def _build(cfg, stop=None):
    nc = bacc.Bacc(
        "TRN2", target_bir_lowering=False, debug=False, num_devices=NCORES
    )
    Gpc = cfg["Gpc"]
    Ppart = cfg["Ppart"]
    Rtot = cfg["Rtot"]
    wrow = cfg["wrow"]
    rg = [list(range(NCORES))]

    inp = {}
    for br in ("c", "s"):
        b = cfg[br]
        nb, totcols, NBp = b["nb"], b["totcols"], b["NBp"]
        inp[f"xperm_{br}"] = nc.dram_tensor(
            f"xperm_{br}", [NBp, F_PAD], F32, kind="ExternalInput"
        )
        inp[f"ell_{br}"] = nc.dram_tensor(
            f"ell_{br}", [128, totcols * 8], I16, kind="ExternalInput"
        )
        inp[f"dst_{br}"] = nc.dram_tensor(
            f"dst_{br}", [128, totcols], BF16, kind="ExternalInput"
        )
        inp[f"dinvp_{br}"] = nc.dram_tensor(
            f"dinvp_{br}", [128, nb], F32, kind="ExternalInput"
        )
        inp[f"pool_{br}"] = nc.dram_tensor(
            f"pool_{br}", [128, nb * Gpc], F32, kind="ExternalInput"
        )
        inp[f"invcnt_{br}"] = nc.dram_tensor(
            f"invcnt_{br}", [Gpc, 1], F32, kind="ExternalInput"
        )
        inp[f"padcnt_{br}"] = nc.dram_tensor(
            f"padcnt_{br}", [64, 1], F32, kind="ExternalInput"
        )
        inp[f"W0_{br}"] = nc.dram_tensor(
            f"W0_{br}", [F_PAD, H], F32, kind="ExternalInput"
        )
        inp[f"W1_{br}"] = nc.dram_tensor(f"W1_{br}", [H, H], F32, kind="ExternalInput")
        for li in (0, 1):
            for nm, shape in (
                (f"b{li}_{br}", [1, H]), (f"b{li}T_{br}", [H, 1]),
                (f"g{li}_{br}", [H, 1]), (f"beta{li}_{br}", [H, 1]),
            ):
                inp[nm] = nc.dram_tensor(nm, shape, F32, kind="ExternalInput")
    inp["Wf1"] = nc.dram_tensor("Wf1", [2 * H, H], F32, kind="ExternalInput")
    inp["bf1"] = nc.dram_tensor("bf1", [1, H], F32, kind="ExternalInput")
    inp["Wf2"] = nc.dram_tensor("Wf2", [H, 2], F32, kind="ExternalInput")
    inp["bf2"] = nc.dram_tensor("bf2", [1, 2], F32, kind="ExternalInput")

    out_t = nc.dram_tensor("out", [Gpc, 2], F32, kind="ExternalOutput")

    part = [nc.dram_tensor(f"part{li}", [Ppart, TROW], BF16) for li in (0, 1)]
    table = [
        nc.dram_tensor(f"table{li}", [Rtot, TROW], BF16, addr_space="Shared")
        for li in (0, 1)
    ]
    st_in = [nc.dram_tensor(f"st{li}_in", [4, H], F32) for li in (0, 1)]
    st_out = [
        nc.dram_tensor(f"st{li}_out", [4, H], F32, addr_space="Shared")
        for li in (0, 1)
    ]

    sec_off = {"c": 0, "s": cfg["c"]["Ppc"]}

    with tile.TileContext(nc, num_cores=NCORES) as tc:
        consts = tc.alloc_tile_pool(name="consts", bufs=1)
        wpool = tc.alloc_tile_pool(name="weights", bufs=1)
        upool = tc.alloc_tile_pool(name="ubuf", bufs=1)
        gpool = tc.alloc_tile_pool(name="gather", bufs=2)
        ohpool = tc.alloc_tile_pool(name="onehot", bufs=2)
        spool = tc.alloc_tile_pool(name="small", bufs=4)
        ppool = tc.alloc_tile_pool(name="psum", bufs=1, space="PSUM")
        xpool = tc.alloc_tile_pool(name="xstage", bufs=3)

        from concourse.masks import make_identity

        ident = consts.tile([128, 128], F32)
        make_identity(nc, ident[:])
        ones_col = consts.tile([128, 1], F32)
        nc.gpsimd.memset(ones_col[:], 1.0)
        ones_row = consts.tile([1, 128], F32)
        nc.gpsimd.memset(ones_row[:], 1.0)
        eps_t = consts.tile([H, 1], F32)
        nc.gpsimd.memset(eps_t[:], EPS)
        zero_big = consts.tile([128, 1024], BF16)
        nc.gpsimd.memset(zero_big[:], 0.0)
        iota_i = consts.tile([128, 128], I32)
        nc.gpsimd.iota(iota_i[:], pattern=[[1, 128]], base=0, channel_multiplier=0)
        iota_bf = consts.tile([128, 128], BF16)
        nc.vector.tensor_copy(out=iota_bf[:], in_=iota_i[:])

        def load_w(name, shape):
            t = wpool.tile(list(shape), F32, tag=name, name=f"w_{name}")
            nc.sync.dma_start(out=t[:], in_=inp[name].ap())
            return t

        def replicate_row(row_ap, width, tag):
            ps = ppool.tile([128, width], F32, tag="ps_u", bufs=3)
            nc.tensor.matmul(
                out=ps[:], lhsT=ones_row[:], rhs=row_ap, start=True, stop=True
            )
            t = wpool.tile([128, width], F32, tag=tag, name=f"rep_{tag}")
            nc.vector.tensor_copy(out=t[:], in_=ps[:])
            return t

        Wt = {}
        for br in ("c", "s"):
            Wt[br, 0] = load_w(f"W0_{br}", (F_PAD, H))
            Wt[br, 1] = load_w(f"W1_{br}", (H, H))
        Wf1 = load_w("Wf1", (2 * H, H))
        Wf2 = load_w("Wf2", (H, 2))
        bT = {}
        gam = {}
        bet = {}
        brep = {}
        for br in ("c", "s"):
            for li in (0, 1):
                bT[br, li] = load_w(f"b{li}T_{br}", (H, 1))
                gam[br, li] = load_w(f"g{li}_{br}", (H, 1))
                bet[br, li] = load_w(f"beta{li}_{br}", (H, 1))
                brow = spool.tile([1, H], F32, tag="brow")
                nc.sync.dma_start(out=brow[:], in_=inp[f"b{li}_{br}"].ap())
                brep[br, li] = replicate_row(brow[:], H, f"brep_{br}{li}")
        bf1row = spool.tile([1, H], F32, tag="brow")
        nc.sync.dma_start(out=bf1row[:], in_=inp["bf1"].ap())
        bf1rep = replicate_row(bf1row[:], H, "bf1rep")
        bf2row = spool.tile([1, 2], F32, tag="brow2")
        nc.sync.dma_start(out=bf2row[:], in_=inp["bf2"].ap())
        bf2rep = replicate_row(bf2row[:], 2, "bf2rep")

        dinvp_t = {}
        padcnt_t = {}
        for br in ("c", "s"):
            nb = cfg[br]["nb"]
            dinvp_t[br] = wpool.tile(
                [128, nb], F32, name=f"dinvp_t_{br}", tag=f"dinvp_{br}"
            )
            nc.sync.dma_start(out=dinvp_t[br][:], in_=inp[f"dinvp_{br}"].ap())
            padcnt_t[br] = wpool.tile(
                [64, 1], F32, name=f"padcnt_t_{br}", tag=f"padcnt_{br}"
            )
            nc.sync.dma_start(out=padcnt_t[br][:], in_=inp[f"padcnt_{br}"].ap())

        # bulk-zero both parts (covers high halves, pad rows, zero blocks)
        for li in (0, 1):
            total = Ppart * TROW
            off = 0
            while off < total:
                n = min(128 * 1024, total - off)
                ncols = n // 128
                nc.sync.dma_start(
                    out=_ap(part[li].ap(), off, [(ncols, 128), (1, ncols)]),
                    in_=zero_big[:, :ncols],
                )
                off += n

        u_t = {}
        acc2_t = {}

        # ------------------------------------------------------------------
        # table section build: rows <- (dinv * src_rows) @ W  (bf16, cols 0:64)
        # ------------------------------------------------------------------
        def build_table(br, li):
            b = cfg[br]
            nb = b["nb"]
            for t in range(nb):
                if li == 0:
                    xt = xpool.tile([128, F_PAD], F32, tag="xt")
                    nc.sync.dma_start(
                        out=xt[:],
                        in_=_ap(inp[f"xperm_{br}"].ap(), t * 128 * F_PAD,
                                [(F_PAD, 128), (1, F_PAD)]),
                    )
                    fin = F_PAD
                else:
                    u = u_t[br]
                    xt = xpool.tile([128, H], F32, tag="xt1")
                    nc.vector.tensor_tensor(
                        out=xt[:], in0=u[:, t * H : (t + 1) * H],
                        in1=_ap(dinvp_t[br], t, [(cfg[br]["nb"], 128), (0, H)]),
                        op=OP.mult,
                    )
                    fin = H
                if li == 0:
                    nc.vector.tensor_tensor(
                        out=xt[:], in0=xt[:],
                        in1=_ap(dinvp_t[br], t, [(nb, 128), (0, F_PAD)]),
                        op=OP.mult,
                    )
                zT_ps = ppool.tile([fin, 128], F32, tag="ps_t", bufs=3)
                nc.tensor.transpose(out=zT_ps[:], in_=xt[:], identity=ident[:])
                zT = xpool.tile([fin, 128], F32, tag=f"zT{fin}")
                nc.vector.tensor_copy(out=zT[:], in_=zT_ps[:])
                r_ps = ppool.tile([128, H], F32, tag="ps_u", bufs=3)
                nc.tensor.matmul(
                    out=r_ps[:], lhsT=zT[:], rhs=Wt[br, li][:], start=True, stop=True
                )
                stage = xpool.tile([128, H], BF16, tag="stage")
                nc.vector.tensor_copy(out=stage[:], in_=r_ps[:])
                nc.sync.dma_start(
                    out=_ap(part[li].ap(), (sec_off[br] + t * 128) * TROW,
                            [(TROW, 128), (1, H)]),
                    in_=stage[:],
                )

        # ------------------------------------------------------------------
        # aggregation: u[:, blk] = sum over windows of one-hot matmuls
        # ------------------------------------------------------------------
        def aggregate(br, li):
            b = cfg[br]
            nb = b["nb"]
            u = u_t.get(br)
            if u is None:
                u = upool.tile([128, nb * H], F32, tag=f"u_{br}", name=f"u_{br}")
                u_t[br] = u
            # expand schedule into per-column (w, blk, first, last)
            colinfo = []
            for (w, blk, cnt) in b["sched"]:
                for i in range(cnt):
                    colinfo.append((w, blk, i == 0, i == cnt - 1))
            ci = 0
            for (w, col0, ncols) in b["calls"]:
                it = gpool.tile([128, SCALL * 8], I16, tag="it")
                nc.sync.dma_start(
                    out=it[:, : ncols * 8],
                    in_=inp[f"ell_{br}"].ap()[:, col0 * 8 : (col0 + ncols) * 8],
                )
                dstc = gpool.tile([128, SCALL], BF16, tag="dstc")
                nc.sync.dma_start(
                    out=dstc[:, :ncols],
                    in_=inp[f"dst_{br}"].ap()[:, col0 : col0 + ncols],
                )
                gbuf = gpool.tile([128, SCALL * TROW], BF16, tag="gbuf")
                num = ncols * 128
                nc.gpsimd.dma_gather(
                    out_ap=_ap(gbuf, 0, [(SCALL * TROW, 128), (TROW, ncols), (1, TROW)]),
                    in_ap=table[li].ap()[w * wrow :, :],
                    idxs_ap=it[:, : ncols * 8],
                    num_idxs=num,
                    num_idxs_reg=num,
                    elem_size=TROW,
                    single_packet=False,
                )
                if AGG_MODE == "gather":
                    ci += ncols
                    continue
                for b0 in range(0, ncols, OHB):
                    bn = min(OHB, ncols - b0)
                    oh = ohpool.tile([128, OHB * 128], BF16, tag="oh")
                    nc.vector.tensor_tensor(
                        out=_ap(oh, 0, [(OHB * 128, 128), (128, bn), (1, 128)]),
                        in0=_ap(dstc, b0, [(SCALL, 128), (1, bn), (0, 128)]),
                        in1=_ap(iota_bf, 0, [(128, 128), (0, bn), (1, 128)]),
                        op=OP.is_equal,
                    )
                    if AGG_MODE == "onehot":
                        ci += bn
                        continue
                    for j in range(bn):
                        cw, cblk, cfirst, clast = colinfo[ci]
                        ci += 1
                        if cfirst:
                            z_ps = ppool.tile(
                                [128, H], F32, tag="ps_z", bufs=2,
                                name=f"zps_{br}{li}_{ci}",
                            )
                            aggregate.cur[br] = z_ps
                        z_ps = aggregate.cur[br]
                        nc.tensor.matmul(
                            out=z_ps[:],
                            lhsT=oh[:, (j + b0 - b0) * 128 : (j + 1) * 128]
                            if False
                            else oh[:, j * 128 : (j + 1) * 128],
                            rhs=_ap(gbuf, (b0 + j) * TROW,
                                    [(SCALL * TROW, 128), (1, H)]),
                            start=cfirst,
                            stop=clast,
                        )
                        if clast:
                            ucol = u[:, cblk * H : (cblk + 1) * H]
                            if cw == 0:
                                nc.vector.tensor_copy(out=ucol, in_=z_ps[:])
                            else:
                                nc.vector.tensor_tensor(
                                    out=ucol, in0=ucol, in1=z_ps[:], op=OP.add
                                )
            assert ci == len(colinfo)
            if AGG_MODE in ("gather", "onehot", "matmul_nou"):
                nc.gpsimd.memset(u[:], 0.0)
            # u = u * dinvp + b
            full = _ap(u, 0, [(nb * H, 128), (H, nb), (1, H)])
            nc.vector.tensor_tensor(
                out=full, in0=full,
                in1=_ap(dinvp_t[br], 0, [(nb, 128), (1, nb), (0, H)]), op=OP.mult,
            )
            nc.vector.tensor_tensor(
                out=full, in0=full,
                in1=_ap(brep[br, li], 0, [(H, 128), (0, nb), (1, H)]), op=OP.add,
            )

        aggregate.cur = {}

        def layer_stats(br, li, st_in_t, row):
            b = cfg[br]
            nb = b["nb"]
            u = u_t[br]
            acc2 = spool.tile([128, H], F32, tag=f"acc2_{br}", name=f"acc2_{br}{li}")
            acc2_t[br] = acc2
            sq = spool.tile([128, H], F32, tag="sq")
            nc.scalar.activation(
                out=sq[:], in_=u[:, 0:H], func=ACT.Square
            )
            nc.vector.tensor_copy(out=acc2[:], in_=sq[:])
            for t in range(1, nb):
                sq = spool.tile([128, H], F32, tag="sq")
                nc.scalar.activation(
                    out=sq[:], in_=u[:, t * H : (t + 1) * H], func=ACT.Square
                )
                nc.vector.tensor_tensor(
                    out=acc2[:], in0=acc2[:], in1=sq[:], op=OP.add
                )
            rsum = spool.tile([128, H], F32, tag="rsum")
            nc.vector.tensor_reduce(
                out=rsum[:], in_=_ap(u, 0, [(nb * H, 128), (1, H), (H, nb)]),
                axis=AX.X, op=OP.add,
            )
            su_ps = ppool.tile([H, 1], F32, tag="ps_t", bufs=3)
            nc.tensor.matmul(
                out=su_ps[:], lhsT=rsum[:], rhs=ones_col[:], start=True, stop=True
            )
            s2_ps = ppool.tile([H, 1], F32, tag="ps_t", bufs=3)
            nc.tensor.matmul(
                out=s2_ps[:], lhsT=acc2[:], rhs=ones_col[:], start=True, stop=True
            )
            corr = spool.tile([H, 1], F32, tag="corr")
            nc.vector.tensor_tensor(
                out=corr[:], in0=padcnt_t[br][:], in1=bT[br, li][:], op=OP.mult
            )
            su = spool.tile([H, 1], F32, tag="su")
            nc.vector.tensor_tensor(out=su[:], in0=su_ps[:], in1=corr[:], op=OP.subtract)
            bsq = spool.tile([H, 1], F32, tag="bsq")
            nc.vector.tensor_tensor(
                out=bsq[:], in0=bT[br, li][:], in1=bT[br, li][:], op=OP.mult
            )
            nc.vector.tensor_tensor(
                out=bsq[:], in0=bsq[:], in1=padcnt_t[br][:], op=OP.mult
            )
            s2 = spool.tile([H, 1], F32, tag="s2")
            nc.vector.tensor_tensor(out=s2[:], in0=s2_ps[:], in1=bsq[:], op=OP.subtract)
            nc.sync.dma_start(out=st_in_t.ap()[row : row + 1, :], in_=su[:])
            nc.sync.dma_start(out=st_in_t.ap()[row + 1 : row + 2, :], in_=s2[:])

        def bn_finish(br, li, st_out_t, row, Ntotal):
            sts = spool.tile([H, 2], F32, tag="sts")
            nc.sync.dma_start(
                out=sts[:], in_=_ap(st_out_t.ap(), row * H, [(1, H), (H, 2)])
            )
            mu = spool.tile([H, 1], F32, tag="mu")
            nc.vector.tensor_scalar_mul(out=mu[:], in0=sts[:, 0:1], scalar1=1.0 / Ntotal)
            ex2 = spool.tile([H, 1], F32, tag="ex2")
            nc.vector.tensor_scalar_mul(out=ex2[:], in0=sts[:, 1:2], scalar1=1.0 / Ntotal)
            musq = spool.tile([H, 1], F32, tag="musq")
            nc.vector.tensor_tensor(out=musq[:], in0=mu[:], in1=mu[:], op=OP.mult)
            var = spool.tile([H, 1], F32, tag="var")
            nc.vector.tensor_tensor(out=var[:], in0=ex2[:], in1=musq[:], op=OP.subtract)
            std = spool.tile([H, 1], F32, tag="std")
            nc.scalar.activation(out=std[:], in_=var[:], func=ACT.Sqrt, bias=eps_t[:])
            istd = spool.tile([H, 1], F32, tag="istd")
            nc.vector.reciprocal(out=istd[:], in_=std[:])
            sc = spool.tile([H, 1], F32, tag="sc")
            nc.vector.tensor_tensor(out=sc[:], in0=gam[br, li][:], in1=istd[:], op=OP.mult)
            sh = spool.tile([H, 1], F32, tag="sh")
            nc.vector.tensor_tensor(out=sh[:], in0=mu[:], in1=sc[:], op=OP.mult)
            nc.vector.tensor_tensor(
                out=sh[:], in0=bet[br, li][:], in1=sh[:], op=OP.subtract
            )
            reps = []
            for vec, tag in ((sc, "screp"), (sh, "shrep")):
                vr_ps = ppool.tile([1, H], F32, tag="ps_t", bufs=3)
                nc.tensor.transpose(out=vr_ps[:], in_=vec[:], identity=ident[:H, :H])
                vr = spool.tile([1, H], F32, tag="vrow")
                nc.vector.tensor_copy(out=vr[:], in_=vr_ps[:])
                reps.append(replicate_row(vr[:], H, f"{tag}_{br}{li}"))
            return reps

        def bn_apply(br, screp, shrep):
            b = cfg[br]
            nb = b["nb"]
            u = u_t[br]
            full = _ap(u, 0, [(nb * H, 128), (H, nb), (1, H)])
            nc.vector.tensor_tensor(
                out=full, in0=full,
                in1=_ap(screp, 0, [(H, 128), (0, nb), (1, H)]), op=OP.mult,
            )
            nc.vector.tensor_tensor(
                out=full, in0=full,
                in1=_ap(shrep, 0, [(H, 128), (0, nb), (1, H)]), op=OP.add,
            )
            flat = u[:, : nb * H]
            nc.scalar.activation(out=flat, in_=flat, func=ACT.Relu)

        # =============================== flow ===============================
        def early_out(src_ap):
            o = spool.tile([Gpc, 2], F32, tag="o_sb")
            nc.vector.tensor_copy(out=o[:], in_=src_ap)
            nc.sync.dma_start(out=out_t.ap(), in_=o[:])

        if stop != "a":
            for br in ("c", "s"):
                build_table(br, 0)
            nc.gpsimd.collective_compute(
                "AllGather", OP.bypass, replica_groups=rg,
                ins=[part[0].ap()], outs=[table[0].ap()],
            )
        if stop == "a":
            tt0 = spool.tile([Gpc, 2], F32, tag="tt0")
            nc.gpsimd.memset(tt0[:], 0.0)
            early_out(tt0[:])
        if stop == "t0":
            tt = spool.tile([Gpc, 2], BF16, tag="tt")
            nc.sync.dma_start(out=tt[:], in_=_ap(table[0].ap(), 0, [(TROW, Gpc), (1, 2)]))
            early_out(tt[:])
        if stop is not None and stop != "a" and stop < "t0":
            pass
        if stop is None or (stop != "a" and stop > "t0"):
            for br in ("c", "s"):
                aggregate(br, 0)
            if stop == "t1":
                early_out(u_t["c"][:Gpc, :2])
        if stop is None or (stop != "a" and stop > "t1"):
            layer_stats("c", 0, st_in[0], 0)
            layer_stats("s", 0, st_in[0], 2)
            nc.gpsimd.collective_compute(
                "AllReduce", OP.add, replica_groups=rg,
                ins=[st_in[0].ap()], outs=[st_out[0].ap()],
            )
            for br, Ntot in (("c", cfg["c"]["N"]), ("s", cfg["s"]["N"])):
                screp, shrep = bn_finish(br, 0, st_out[0], 0 if br == "c" else 2, Ntot)
                bn_apply(br, screp, shrep)
                build_table(br, 1)
            if stop == "t2":
                early_out(u_t["c"][:Gpc, :2])
        if stop is None or (stop != "a" and stop > "t2"):
            nc.gpsimd.collective_compute(
                "AllGather", OP.bypass, replica_groups=rg,
                ins=[part[1].ap()], outs=[table[1].ap()],
            )
            for br in ("c", "s"):
                aggregate(br, 1)
            if stop == "t3":
                early_out(u_t["c"][:Gpc, :2])
        if stop is None or (stop != "a" and stop > "t3"):
            layer_stats("c", 1, st_in[1], 0)
            layer_stats("s", 1, st_in[1], 2)
            nc.gpsimd.collective_compute(
                "AllReduce", OP.add, replica_groups=rg,
                ins=[st_in[1].ap()], outs=[st_out[1].ap()],
            )
        hcat = spool.tile([Gpc, 2 * H], F32, tag="hcat")
        tail = [] if stop is not None else [
            ("c", cfg["c"]["N"]), ("s", cfg["s"]["N"])]
        for br, Ntot in tail:
            screp, shrep = bn_finish(br, 1, st_out[1], 0 if br == "c" else 2, Ntot)
            bn_apply(br, screp, shrep)
            b = cfg[br]
            nb = b["nb"]
            u = u_t[br]
            pool_ps = ppool.tile(
                [Gpc, H], F32, tag="ps_u", bufs=3, name=f"pool_ps_{br}"
            )
            for t in range(nb):
                poh = gpool.tile([128, Gpc], F32, tag="poh")
                nc.sync.dma_start(
                    out=poh[:],
                    in_=inp[f"pool_{br}"].ap()[:, t * Gpc : (t + 1) * Gpc],
                )
                nc.tensor.matmul(
                    out=pool_ps[:], lhsT=poh[:], rhs=u[:, t * H : (t + 1) * H],
                    start=(t == 0), stop=(t == nb - 1),
                )
            ic = spool.tile([Gpc, 1], F32, tag="ic")
            nc.sync.dma_start(out=ic[:], in_=inp[f"invcnt_{br}"].ap())
            col0 = 0 if br == "c" else H
            nc.vector.tensor_tensor(
                out=hcat[:, col0 : col0 + H], in0=pool_ps[:],
                in1=_ap(ic, 0, [(1, Gpc), (0, H)]), op=OP.mult,
            )

        if stop is not None:
            hcT_ps = None
        else:
            hcT_ps = ppool.tile([2 * H, Gpc], F32, tag="ps_t", bufs=3)
            nc.tensor.transpose(out=hcT_ps[:], in_=hcat[:], identity=ident[:Gpc, :Gpc])
        if stop is None:
            hcT = spool.tile([2 * H, Gpc], F32, tag="hcT")
            nc.vector.tensor_copy(out=hcT[:], in_=hcT_ps[:])
            f_ps = ppool.tile([Gpc, H], F32, tag="ps_u", bufs=3)
            nc.tensor.matmul(out=f_ps[:], lhsT=hcT[:], rhs=Wf1[:], start=True, stop=True)
            f_sb = spool.tile([Gpc, H], F32, tag="f_sb")
            nc.vector.tensor_tensor(out=f_sb[:], in0=f_ps[:], in1=bf1rep[:Gpc, :], op=OP.add)
            nc.scalar.activation(out=f_sb[:], in_=f_sb[:], func=ACT.Relu)
            fT_ps = ppool.tile([H, Gpc], F32, tag="ps_t", bufs=3)
            nc.tensor.transpose(out=fT_ps[:], in_=f_sb[:], identity=ident[:Gpc, :Gpc])
            fT = spool.tile([H, Gpc], F32, tag="fT")
            nc.vector.tensor_copy(out=fT[:], in_=fT_ps[:])
            o_ps = ppool.tile([Gpc, 2], F32, tag="ps_u", bufs=3)
            nc.tensor.matmul(out=o_ps[:], lhsT=fT[:], rhs=Wf2[:], start=True, stop=True)
            o_sb = spool.tile([Gpc, 2], F32, tag="o_sb")
            nc.vector.tensor_tensor(out=o_sb[:], in0=o_ps[:], in1=bf2rep[:Gpc, :], op=OP.add)
            nc.sync.dma_start(out=out_t.ap(), in_=o_sb[:])

        for p in (xpool, ppool, spool, ohpool, gpool, upool, wpool, consts):
            p.release()

    nc.compile()
    return nc


def kernel(_G=G_DEFAULT, _trace=False, _return_results=False, _stop=None, **inputs):
    cfg, in_maps = _prep(inputs, _G, NCORES)
    nc = _build(cfg, stop=_stop)
    res = run_bass_kernel_spmd(
        nc, in_maps, core_ids=list(range(NCORES)), trace=_trace
    )
    out = np.concatenate([res.results[k]["out"] for k in range(NCORES)], axis=0)
    if _return_results:
        return out, res
    return out
